# revision 8
# baseline (speedup 1.0000x reference)
"""Correlation module kernel for 8 TRN2 NeuronCores.

Reference computation (per batch element n, pure data-parallel over N):
    A_n = X_n @ U_n^T / sqrt(D)          # [L, O]
    W_n = sigmoid(A_n) - 0.5             # = 0.5 * tanh(A_n / 2)
    F_n = W_n @ U_n                      # [L, D]

Shapes: x [L=512, N=64, D=512] f32, upfold [O=512, N=64, D=512] f32.
Sharding: N axis across 8 cores (8 batch elements per core), no comms.

Device kernel (per core, per n):
    MM1:  psum_AT[o, l] = sum_d uT[d, o] * xT[d, l]      (fp16 in, f32 acc)
    ACT:  w[o, l] = tanh(psum_AT * 1/(2*sqrt(D)))        (-> fp16)
    MM2:  psum_F[l, d] = sum_o w[o, l] * (0.5*u)[o, d]   (fp16 in, f32 acc)
    DVE:  f[l, d] = psum_F                               (-> fp16)
    DMA out to y[l, n, d]; host upcasts to f32.

Host pre-arranges per-core inputs as fp16 in the exact layouts the PE
needs (d-major for MM1 operands, o-major for MM2's moving operand), so
the device does zero transposes and minimum HBM traffic (16.8 MB/core).

Schedule notes (measured on HW via neuron-profile):
 - The PE clock ramp is TIME-based: full speed arrives ~4.9us after the
   first sustained PE activity (which can't start before the framework
   preamble barrier at ~7us).  10 dummy warm-up matmuls on scratch SBUF
   bridge exactly to the first loads' arrival (~10.7us); any idle gap in
   PE activity RESETS the ramp (a 3.4us gap costs ~5us), so warm-ups
   must not undershoot.  A dummy tanh pre-triggers the ACT table load.
 - n0's xt/ut first halves ride separate HWDGE rings (sync + scalar) so
   their transfers overlap; per-DMA-queue bandwidth is only ~150 GB/s
   (HWDGE) / ~50 GB/s (SWDGE), far below the 358 GB/s core aggregate,
   so big loads must be spread across queues/rings to pipeline.
 - MM1 runs d-major over 4 PSUM banks then closes each o-block early so
   its tanh overlaps remaining matmuls; MM2 runs o-major over 4 more
   banks.  Steady state: 216 ns/matmul (512 rows @ 2.4 GHz, LDWEIGHTS
   fully hidden) — the hard PE floor.  Some runs are HAM/GPIO power
   throttled to 13/16 duty (262.6 ns pitch); that is environmental.
 - Stores for n0-n6 ride the gpsimd SWDGE ring spread over 4 queues
   (num_swdge_queues=4 — a single queue backlogs at ~50 GB/s).  The
   last element closes its final two l-blocks in d-halves, each in its
   own PSUM tile borrowed from the freed MM1 banks (per-tile dependency
   tracking lets each half's cast+store overlap remaining matmuls), and
   all its stores ride the two HWDGE rings in <=64 KB chunks.  DMA
   completion semaphores land ~1.5us after issue; exec_time is measured
   to the literal last instruction (incl. a fixed ~7us framework
   semaphore-clear epilogue), so the tail chain cast->issue->completion
   after the last matmul is what matters.
"""

import numpy as np

L, O, N, D = 512, 512, 64, 512
NCORES = 8
NLOC = N // NCORES  # 8 batch elements per core
P = 128  # SBUF partitions
DB = D // P  # 4 d-blocks
OB = O // P  # 4 o-blocks
LB = L // P  # 4 l-blocks
# Bridges PE activity from tile-body start (~7.4us) to the first
# loads' completion sem (~9.3us with 128KB-granular first chunks
# interleaved on both HWDGE rings) at the cold 427ns/MM rate.  Too few
# warm-ups opens a PE idle gap; a sub-us gap does not trip the HAM MID
# window (4096 cycles ~3.4us of idleness), so a small undershoot is
# tolerable, while each excess warm-up delays the first real matmul by
# 427ns once loads have landed.
WARMUP_MMS = 5

_cache = {}


def _build_program():
    import concourse.bass as bass
    import concourse.mybir as mybir
    import concourse.tile as tile
    from concourse import bacc

    FP16 = mybir.dt.float16
    F32 = mybir.dt.float32
    Tanh = mybir.ActivationFunctionType.Tanh
    Copy = mybir.ActivationFunctionType.Copy

    nc = bacc.Bacc(
        "TRN2", target_bir_lowering=False, debug=False, num_swdge_queues=4
    )
    xt_d = nc.declare_dram_parameter("xt", [NLOC, D, L], FP16, isOutput=False)
    ut_d = nc.declare_dram_parameter("ut", [NLOC, D, O], FP16, isOutput=False)
    un_d = nc.declare_dram_parameter("un", [NLOC, O, D], FP16, isOutput=False)
    # y is [NLOC, L, D] (contiguous 512KB per batch element) so store DMA
    # descriptors write fully contiguous DRAM ranges; the host transposes
    # back to [L, N, D] during the fp32 upcast it does anyway.
    y_d = nc.declare_dram_parameter("y", [NLOC, L, D], FP16, isOutput=True)

    s2 = 1.0 / (2.0 * float(np.sqrt(D)))  # tanh half-argument scale

    with tile.TileContext(nc) as tc:
        with (
            tc.tile_pool(name="xt", bufs=NLOC) as xt_pool,
            tc.tile_pool(name="ut", bufs=NLOC) as ut_pool,
            tc.tile_pool(name="un", bufs=NLOC) as un_pool,
            tc.tile_pool(name="w", bufs=2) as w_pool,
            tc.tile_pool(name="fo", bufs=2) as f_pool,
            tc.tile_pool(name="scr", bufs=1) as scr_pool,
            tc.tile_pool(name="psa", bufs=1, space="PSUM") as psa_pool,
            tc.tile_pool(name="psf", bufs=1, space="PSUM") as psf_pool,
        ):
            # PE warm-up: dense dummy matmuls on (uninitialized) scratch
            # while the first loads are still in flight. Drives the HAM
            # activity window so real matmuls start at full clock.
            scr_t = scr_pool.tile([P, L], FP16, tag="scr")
            nc.gpsimd.memset(scr_t[:], 0.0)
            # trigger the ACT tanh table load now (1.3us) so the first real
            # tanh doesn't stall the MM1->MM2 pipeline on it
            scr2_t = scr_pool.tile([P, 1], FP16, tag="scr2")
            nc.scalar.activation(scr2_t[:], scr_t[:, 0:1], Tanh, scale=s2)
            ps_w = psa_pool.tile([P, L], F32, tag="psa0", name="ps_warm")
            for _ in range(WARMUP_MMS):
                nc.tensor.matmul(
                    ps_w[:], lhsT=scr_t[:, :P], rhs=scr_t[:], start=True, stop=True
                )

            for n in range(NLOC):
                # -- loads: xt halves + un on sync ring, ut halves on scalar --
                xt_t = xt_pool.tile([P, DB, L], FP16, tag="xt")
                ut_t = ut_pool.tile([P, DB, O], FP16, tag="ut")
                un_t = un_pool.tile([P, OB, D], FP16, tag="un")
                xt_ap = xt_d[n].rearrange("(b p) l -> p b l", p=P)
                ut_ap = ut_d[n].rearrange("(b p) o -> p b o", p=P)
                un_ap = un_d[n].rearrange("(b p) d -> p b d", p=P)
                if n == 0:
                    # first element: 128KB-granular chunks.  The db0 operand
                    # pair (ut[0], xt[0]) rides the sync ring back-to-back —
                    # its HWDGE queue starts streaming ~1.6us after issue vs
                    # ~2.3us for the scalar ring — so MM1 can start ~9.3us,
                    # ~2.5us earlier than with half-tensor chunks.  The db1
                    # pair and the remainder interleave across both rings.
                    nc.sync.dma_start(ut_t[:, 0, :], ut_ap[:, 0, :])
                    nc.scalar.dma_start(ut_t[:, 1, :], ut_ap[:, 1, :])
                    nc.sync.dma_start(xt_t[:, 0, :], xt_ap[:, 0, :])
                    nc.scalar.dma_start(xt_t[:, 1, :], xt_ap[:, 1, :])
                    nc.sync.dma_start(ut_t[:, 2:4, :], ut_ap[:, 2:4, :])
                    nc.scalar.dma_start(xt_t[:, 2:4, :], xt_ap[:, 2:4, :])
                    nc.sync.dma_start(un_t[:, 0:2, :], un_ap[:, 0:2, :])
                    nc.scalar.dma_start(un_t[:, 2:4, :], un_ap[:, 2:4, :])
                else:
                    nc.sync.dma_start(xt_t[:, 0:2, :], xt_ap[:, 0:2, :])
                    nc.sync.dma_start(ut_t[:, 0:2, :], ut_ap[:, 0:2, :])
                    nc.sync.dma_start(xt_t[:, 2:4, :], xt_ap[:, 2:4, :])
                    nc.sync.dma_start(ut_t[:, 2:4, :], ut_ap[:, 2:4, :])
                    nc.sync.dma_start(un_t[:], un_ap)

                # -- MM1 d-major over 4 PSUM banks: AT[o,l] += uT.T @ xT --
                ps_a = [
                    psa_pool.tile([P, L], F32, tag=f"psa{ob}", name=f"ps_a{ob}")
                    for ob in range(OB)
                ]
                # d-major for the first two d-blocks (compute starts as soon
                # as the first load halves land), then o-major so each
                # o-block's accumulation closes early and its tanh overlaps
                # the remaining matmuls.
                mm1_order = [(db, ob) for db in range(2) for ob in range(OB)]
                mm1_order += [(db, ob) for ob in range(OB) for db in range(2, DB)]
                for db, ob in mm1_order:
                    nc.tensor.matmul(
                        ps_a[ob][:],
                        lhsT=ut_t[:, db, bass.ts(ob, P)],
                        rhs=xt_t[:, db, :],
                        start=(db == 0),
                        stop=(db == DB - 1),
                    )
                # -- sigmoid-center: w = tanh(AT * s2)  (fp16) --
                w_t = w_pool.tile([P, OB, L], FP16, tag="w")
                for ob in range(OB):
                    nc.scalar.activation(w_t[:, ob, :], ps_a[ob][:], Tanh, scale=s2)

                # -- MM2 o-major over 4 PSUM banks: F[l,d] += w.T @ un --
                ps_f = [
                    psf_pool.tile([P, D], F32, tag=f"psf{lb}", name=f"ps_f{lb}")
                    for lb in range(LB)
                ]
                # o-major lets MM2 start with just w[0] ready; for the last
                # batch element close each l-block early (lb-major) so the
                # casts/stores pipeline during the final matmuls.
                last = n == NLOC - 1
                if last:
                    mm2_order = [
                        (ob, lb) for lb in range(LB - 2) for ob in range(OB)
                    ]
                else:
                    mm2_order = [(ob, lb) for ob in range(OB) for lb in range(LB)]
                for ob, lb in mm2_order:
                    nc.tensor.matmul(
                        ps_f[lb][:],
                        lhsT=w_t[:, ob, bass.ts(lb, P)],
                        rhs=un_t[:, ob, :],
                        start=(ob == 0),
                        stop=(ob == OB - 1),
                    )
                ps_h = None
                if last:
                    # the last two l-blocks close in d-halves, each half in
                    # its OWN psum tile (reusing MM1's by-now-free psa banks)
                    # so the per-tile dependency tracker lets each half's
                    # cast+store overlap the remaining matmuls.  Smaller
                    # chunks matter here because a single DMA queue moves
                    # only ~50 GB/s (1 KB packets): a full 128 KB l-block
                    # store costs 2.6 us of queue time after its cast.
                    h = D // 2
                    ps_h = [
                        psa_pool.tile([P, h], F32, tag=f"psa{i}", name=f"ps_h{i}")
                        for i in range(4)
                    ]
                    for i in range(4):
                        lb = LB - 2 + i // 2
                        for ob in range(OB):
                            nc.tensor.matmul(
                                ps_h[i][:],
                                lhsT=w_t[:, ob, bass.ts(lb, P)],
                                rhs=un_t[:, ob, (i % 2) * h : (i % 2 + 1) * h],
                                start=(ob == 0),
                                stop=(ob == OB - 1),
                            )
                # -- PSUM -> SBUF casts split across DVE and ACT; store each
                # quarter via SWDGE as soon as its cast lands (gpsimd is
                # otherwise idle; keeps DMA issue off the ACT/Sync rings) --
                f_t = f_pool.tile([P, LB, D], FP16, tag="f")
                y_ap = y_d[n].rearrange("(b p) d -> p b d", p=P)
                for lb in range(LB):
                    if last:
                        # tail-critical: chunks are ordered so the HWDGE
                        # rings have NO queued backlog when the last-closing
                        # chunks arrive — lb0 (closes ~2.6us before the last
                        # matmul) rides the otherwise-idle SWDGE, lb1/h1/q_a
                        # ride sync, h0/h2/q_b ride scalar, each issued as
                        # soon as its psum tile's cast lands so the final
                        # 32KB quarters see empty queues.
                        h = D // 2
                        q = D // 4
                        if lb == 0:
                            nc.vector.tensor_copy(f_t[:, lb, :], ps_f[lb][:])
                            nc.gpsimd.dma_start(y_ap[:, lb, :], f_t[:, lb, :])
                        elif lb == 1:
                            nc.scalar.activation(f_t[:, lb, :], ps_f[lb][:], Copy)
                            nc.sync.dma_start(y_ap[:, lb, :], f_t[:, lb, :])
                        elif lb == 2:
                            nc.vector.tensor_copy(f_t[:, lb, 0:h], ps_h[0][:])
                            nc.scalar.dma_start(y_ap[:, lb, 0:h], f_t[:, lb, 0:h])
                            nc.vector.tensor_copy(f_t[:, lb, h:D], ps_h[1][:])
                            nc.sync.dma_start(y_ap[:, lb, h:D], f_t[:, lb, h:D])
                        else:
                            nc.vector.tensor_copy(f_t[:, lb, 0:h], ps_h[2][:])
                            nc.scalar.dma_start(y_ap[:, lb, 0:h], f_t[:, lb, 0:h])
                            nc.vector.tensor_copy(
                                f_t[:, lb, h : h + q], ps_h[3][:, 0:q]
                            )
                            nc.sync.dma_start(
                                y_ap[:, lb, h : h + q], f_t[:, lb, h : h + q]
                            )
                            nc.scalar.activation(
                                f_t[:, lb, h + q : D], ps_h[3][:, q:h], Copy
                            )
                            nc.scalar.dma_start(
                                y_ap[:, lb, h + q : D], f_t[:, lb, h + q : D]
                            )
                        continue
                    if lb % 2 == 0:
                        nc.vector.tensor_copy(f_t[:, lb, :], ps_f[lb][:])
                    else:
                        nc.scalar.activation(f_t[:, lb, :], ps_f[lb][:], Copy)
                    nc.gpsimd.dma_start(y_ap[:, lb, :], f_t[:, lb, :])
    nc.compile()
    return nc


def _prepare_in_maps(x, u):
    f16 = np.float16
    in_maps = []
    for c in range(NCORES):
        ns = slice(c * NLOC, (c + 1) * NLOC)
        xs = x[:, ns, :]  # [L, NLOC, D]
        us = u[:, ns, :]  # [O, NLOC, D]
        in_maps.append(
            {
                # X^T per n: [NLOC, D, L]
                "xt": np.ascontiguousarray(xs.transpose(1, 2, 0)).astype(f16),
                # U^T per n: [NLOC, D, O]
                "ut": np.ascontiguousarray(us.transpose(1, 2, 0)).astype(f16),
                # U natural per n, pre-scaled by 0.5 (folds sigmoid's -0.5
                # via sigmoid(a)-0.5 = 0.5*tanh(a/2)): [NLOC, O, D]
                "un": (0.5 * us.transpose(1, 0, 2)).astype(f16),
            }
        )
    return in_maps


def _run(inputs, trace=False, **spmd_kwargs):
    from concourse.bass_utils import run_bass_kernel_spmd

    x = np.asarray(inputs["x"], dtype=np.float32)
    u = np.asarray(inputs["upfold"], dtype=np.float32)
    assert x.shape == (L, N, D) and u.shape == (O, N, D)

    if "nc" not in _cache:
        _cache["nc"] = _build_program()
    nc = _cache["nc"]

    in_maps = _prepare_in_maps(x, u)
    res = run_bass_kernel_spmd(
        nc, in_maps, core_ids=list(range(NCORES)), trace=trace, **spmd_kwargs
    )
    # device y is [NLOC, L, D]; transpose back while assembling [L, N, D]
    out = np.concatenate(
        [r["y"].transpose(1, 0, 2) for r in res.results], axis=1
    )
    return np.ascontiguousarray(out.astype(np.float32)), res


def kernel(**inputs) -> np.ndarray:
    out, _ = _run(inputs, trace=False)
    return out



# revision 11
# speedup vs baseline: 1.0384x; 1.0384x over previous
"""Correlation module kernel for 8 TRN2 NeuronCores.

Reference computation (per batch element n, pure data-parallel over N):
    A_n = X_n @ U_n^T / sqrt(D)          # [L, O]
    W_n = sigmoid(A_n) - 0.5             # = 0.5 * tanh(A_n / 2)
    F_n = W_n @ U_n                      # [L, D]

Shapes: x [L=512, N=64, D=512] f32, upfold [O=512, N=64, D=512] f32.
Sharding: N axis across 8 cores (8 batch elements per core), no comms.

Device kernel (per core, per n):
    MM1:  psum_AT[o, l] = sum_d uT[d, o] * xT[d, l]      (fp16 in, f32 acc)
    ACT:  w[o, l] = tanh(psum_AT * 1/(2*sqrt(D)))        (-> fp16)
    MM2:  psum_F[l, d] = sum_o w[o, l] * (0.5*u)[o, d]   (fp16 in, f32 acc)
    DVE:  f[l, d] = psum_F                               (-> fp16)
    DMA out to y[l, n, d]; host upcasts to f32.

Host pre-arranges per-core inputs as fp16 in the exact layouts the PE
needs (d-major for MM1 operands, o-major for MM2's moving operand), so
the device does zero transposes and minimum HBM traffic (16.8 MB/core).

Schedule notes (measured on HW via neuron-profile):
 - The PE clock ramp is TIME-based: full speed arrives ~4.9us after the
   first sustained PE activity (which can't start before the framework
   preamble barrier at ~7us).  10 dummy warm-up matmuls on scratch SBUF
   bridge exactly to the first loads' arrival (~10.7us); any idle gap in
   PE activity RESETS the ramp (a 3.4us gap costs ~5us), so warm-ups
   must not undershoot.  A dummy tanh pre-triggers the ACT table load.
 - n0's xt/ut first halves ride separate HWDGE rings (sync + scalar) so
   their transfers overlap; per-DMA-queue bandwidth is only ~150 GB/s
   (HWDGE) / ~50 GB/s (SWDGE), far below the 358 GB/s core aggregate,
   so big loads must be spread across queues/rings to pipeline.
 - MM1 runs d-major over 4 PSUM banks then closes each o-block early so
   its tanh overlaps remaining matmuls; MM2 runs o-major over 4 more
   banks.  Steady state: 216 ns/matmul (512 rows @ 2.4 GHz, LDWEIGHTS
   fully hidden) — the hard PE floor.  Some runs are HAM/GPIO power
   throttled to 13/16 duty (262.6 ns pitch); that is environmental.
 - Stores for n0-n6 ride the gpsimd SWDGE ring spread over 4 queues
   (num_swdge_queues=4 — a single queue backlogs at ~50 GB/s).  The
   last element closes its final two l-blocks in d-halves, each in its
   own PSUM tile borrowed from the freed MM1 banks (per-tile dependency
   tracking lets each half's cast+store overlap remaining matmuls), and
   all its stores ride the two HWDGE rings in <=64 KB chunks.  DMA
   completion semaphores land ~1.5us after issue; exec_time is measured
   to the literal last instruction (incl. a fixed ~7us framework
   semaphore-clear epilogue), so the tail chain cast->issue->completion
   after the last matmul is what matters.
"""

import numpy as np

L, O, N, D = 512, 512, 64, 512
NCORES = 8
NLOC = N // NCORES  # 8 batch elements per core
P = 128  # SBUF partitions
DB = D // P  # 4 d-blocks
OB = O // P  # 4 o-blocks
LB = L // P  # 4 l-blocks
# Bridges PE activity from tile-body start (~7.4us) to the first
# loads' completion sem (~9.6us with the db0 operand pair on parallel
# HWDGE rings).  Warm-ups are FD=256 (213ns at the cold 1.2GHz clock)
# so the bridge quantization — the idle sliver between the last warm-up
# and the first real matmul, and the cost of each excess warm-up — is
# half that of FD=512 warm-ups.  A macroscopic idle gap here RESETS the
# HAM clock ramp (measured: a 2us gap kept 12 real matmuls at the cold
# 427ns rate, ~+2.5us), so the bridge must not undershoot.
WARMUP_MMS = 11
WARMUP_FD = L // 2

_cache = {}


def _build_program():
    import concourse.bass as bass
    import concourse.mybir as mybir
    import concourse.tile as tile
    from concourse import bacc

    FP16 = mybir.dt.float16
    F32 = mybir.dt.float32
    Tanh = mybir.ActivationFunctionType.Tanh
    Copy = mybir.ActivationFunctionType.Copy

    nc = bacc.Bacc(
        "TRN2", target_bir_lowering=False, debug=False, num_swdge_queues=4
    )
    xt_d = nc.declare_dram_parameter("xt", [NLOC, D, L], FP16, isOutput=False)
    ut_d = nc.declare_dram_parameter("ut", [NLOC, D, O], FP16, isOutput=False)
    un_d = nc.declare_dram_parameter("un", [NLOC, O, D], FP16, isOutput=False)
    # y is [NLOC, L, D] (contiguous 512KB per batch element) so store DMA
    # descriptors write fully contiguous DRAM ranges; the host transposes
    # back to [L, N, D] during the fp32 upcast it does anyway.
    y_d = nc.declare_dram_parameter("y", [NLOC, L, D], FP16, isOutput=True)

    s2 = 1.0 / (2.0 * float(np.sqrt(D)))  # tanh half-argument scale

    with tile.TileContext(nc) as tc:
        with (
            tc.tile_pool(name="xt", bufs=NLOC) as xt_pool,
            tc.tile_pool(name="ut", bufs=NLOC) as ut_pool,
            tc.tile_pool(name="un", bufs=NLOC) as un_pool,
            tc.tile_pool(name="w", bufs=2) as w_pool,
            tc.tile_pool(name="fo", bufs=2) as f_pool,
            tc.tile_pool(name="scr", bufs=1) as scr_pool,
            tc.tile_pool(name="psa", bufs=1, space="PSUM") as psa_pool,
            tc.tile_pool(name="psf", bufs=1, space="PSUM") as psf_pool,
        ):
            # PE warm-up: dense dummy matmuls on (uninitialized) scratch
            # while the first loads are still in flight. Drives the HAM
            # activity window so real matmuls start at full clock.
            scr_t = scr_pool.tile([P, L], FP16, tag="scr")
            nc.gpsimd.memset(scr_t[:], 0.0)
            # trigger the ACT tanh table load now (1.3us) so the first real
            # tanh doesn't stall the MM1->MM2 pipeline on it
            scr2_t = scr_pool.tile([P, 1], FP16, tag="scr2")
            nc.scalar.activation(scr2_t[:], scr_t[:, 0:1], Tanh, scale=s2)
            ps_w = psa_pool.tile([P, L], F32, tag="psa0", name="ps_warm")
            for _ in range(WARMUP_MMS):
                nc.tensor.matmul(
                    ps_w[:, :WARMUP_FD],
                    lhsT=scr_t[:, :P],
                    rhs=scr_t[:, :WARMUP_FD],
                    start=True,
                    stop=True,
                )

            for n in range(NLOC):
                # -- loads: xt halves + un on sync ring, ut halves on scalar --
                xt_t = xt_pool.tile([P, DB, L], FP16, tag="xt")
                ut_t = ut_pool.tile([P, DB, O], FP16, tag="ut")
                un_t = un_pool.tile([P, OB, D], FP16, tag="un")
                xt_ap = xt_d[n].rearrange("(b p) l -> p b l", p=P)
                ut_ap = ut_d[n].rearrange("(b p) o -> p b o", p=P)
                un_ap = un_d[n].rearrange("(b p) d -> p b d", p=P)
                if n == 0:
                    # first element: 128KB-granular chunks, with the db0
                    # operand pair (ut[0], xt[0]) riding the two HWDGE rings
                    # IN PARALLEL.  Early in the kernel all 8 cores burst
                    # their first loads simultaneously, so each queue only
                    # sustains ~140 GB/s (128KB ~ 0.93us); parallel rings get
                    # the pair in by ~9.5us vs ~11.9us for half-tensor
                    # chunks.  db1's pair follows on the same split, then the
                    # d-block remainders and un.
                    nc.sync.dma_start(ut_t[:, 0, :], ut_ap[:, 0, :])
                    nc.scalar.dma_start(xt_t[:, 0, :], xt_ap[:, 0, :])
                    nc.sync.dma_start(ut_t[:, 1, :], ut_ap[:, 1, :])
                    nc.scalar.dma_start(xt_t[:, 1, :], xt_ap[:, 1, :])
                    nc.sync.dma_start(ut_t[:, 2:4, :], ut_ap[:, 2:4, :])
                    nc.scalar.dma_start(xt_t[:, 2:4, :], xt_ap[:, 2:4, :])
                    nc.sync.dma_start(un_t[:, 0:2, :], un_ap[:, 0:2, :])
                    nc.scalar.dma_start(un_t[:, 2:4, :], un_ap[:, 2:4, :])
                else:
                    nc.sync.dma_start(xt_t[:, 0:2, :], xt_ap[:, 0:2, :])
                    nc.sync.dma_start(ut_t[:, 0:2, :], ut_ap[:, 0:2, :])
                    nc.sync.dma_start(xt_t[:, 2:4, :], xt_ap[:, 2:4, :])
                    nc.sync.dma_start(ut_t[:, 2:4, :], ut_ap[:, 2:4, :])
                    nc.sync.dma_start(un_t[:], un_ap)

                # -- MM1 d-major over 4 PSUM banks: AT[o,l] += uT.T @ xT --
                ps_a = [
                    psa_pool.tile([P, L], F32, tag=f"psa{ob}", name=f"ps_a{ob}")
                    for ob in range(OB)
                ]
                # d-major for the first two d-blocks (compute starts as soon
                # as the first load halves land), then o-major so each
                # o-block's accumulation closes early and its tanh overlaps
                # the remaining matmuls.
                mm1_order = [(db, ob) for db in range(2) for ob in range(OB)]
                mm1_order += [(db, ob) for ob in range(OB) for db in range(2, DB)]
                for db, ob in mm1_order:
                    nc.tensor.matmul(
                        ps_a[ob][:],
                        lhsT=ut_t[:, db, bass.ts(ob, P)],
                        rhs=xt_t[:, db, :],
                        start=(db == 0),
                        stop=(db == DB - 1),
                    )
                # -- sigmoid-center: w = tanh(AT * s2)  (fp16) --
                w_t = w_pool.tile([P, OB, L], FP16, tag="w")
                for ob in range(OB):
                    nc.scalar.activation(w_t[:, ob, :], ps_a[ob][:], Tanh, scale=s2)

                # -- MM2 o-major over 4 PSUM banks: F[l,d] += w.T @ un --
                ps_f = [
                    psf_pool.tile([P, D], F32, tag=f"psf{lb}", name=f"ps_f{lb}")
                    for lb in range(LB)
                ]
                # o-major lets MM2 start with just w[0] ready; for the last
                # batch element close each l-block early (lb-major) so the
                # casts/stores pipeline during the final matmuls.
                last = n == NLOC - 1
                if last:
                    mm2_order = [
                        (ob, lb) for lb in range(LB - 2) for ob in range(OB)
                    ]
                else:
                    mm2_order = [(ob, lb) for ob in range(OB) for lb in range(LB)]
                for ob, lb in mm2_order:
                    nc.tensor.matmul(
                        ps_f[lb][:],
                        lhsT=w_t[:, ob, bass.ts(lb, P)],
                        rhs=un_t[:, ob, :],
                        start=(ob == 0),
                        stop=(ob == OB - 1),
                    )
                ps_h = None
                if last:
                    # the last two l-blocks close in d-halves, each half in
                    # its OWN psum tile (reusing MM1's by-now-free psa banks)
                    # so the per-tile dependency tracker lets each half's
                    # cast+store overlap the remaining matmuls.  Smaller
                    # chunks matter here because a single DMA queue moves
                    # only ~50 GB/s (1 KB packets): a full 128 KB l-block
                    # store costs 2.6 us of queue time after its cast.
                    h = D // 2
                    ps_h = [
                        psa_pool.tile([P, h], F32, tag=f"psa{i}", name=f"ps_h{i}")
                        for i in range(4)
                    ]
                    for i in range(4):
                        lb = LB - 2 + i // 2
                        for ob in range(OB):
                            nc.tensor.matmul(
                                ps_h[i][:],
                                lhsT=w_t[:, ob, bass.ts(lb, P)],
                                rhs=un_t[:, ob, (i % 2) * h : (i % 2 + 1) * h],
                                start=(ob == 0),
                                stop=(ob == OB - 1),
                            )
                # -- PSUM -> SBUF casts split across DVE and ACT; store each
                # quarter via SWDGE as soon as its cast lands (gpsimd is
                # otherwise idle; keeps DMA issue off the ACT/Sync rings) --
                f_t = f_pool.tile([P, LB, D], FP16, tag="f")
                y_ap = y_d[n].rearrange("(b p) d -> p b d", p=P)
                for lb in range(LB):
                    if last:
                        # tail-critical: chunks are ordered so the HWDGE
                        # rings have NO queued backlog when the last-closing
                        # chunks arrive — lb0 (closes ~2.6us before the last
                        # matmul) rides the otherwise-idle SWDGE, lb1/h1/q_a
                        # ride sync, h0/h2/q_b ride scalar, each issued as
                        # soon as its psum tile's cast lands so the final
                        # 32KB quarters see empty queues.
                        h = D // 2
                        q = D // 4
                        if lb == 0:
                            nc.vector.tensor_copy(f_t[:, lb, :], ps_f[lb][:])
                            nc.gpsimd.dma_start(y_ap[:, lb, :], f_t[:, lb, :])
                        elif lb == 1:
                            nc.scalar.activation(f_t[:, lb, :], ps_f[lb][:], Copy)
                            nc.sync.dma_start(y_ap[:, lb, :], f_t[:, lb, :])
                        elif lb == 2:
                            nc.vector.tensor_copy(f_t[:, lb, 0:h], ps_h[0][:])
                            nc.scalar.dma_start(y_ap[:, lb, 0:h], f_t[:, lb, 0:h])
                            nc.vector.tensor_copy(f_t[:, lb, h:D], ps_h[1][:])
                            nc.sync.dma_start(y_ap[:, lb, h:D], f_t[:, lb, h:D])
                        else:
                            nc.vector.tensor_copy(f_t[:, lb, 0:h], ps_h[2][:])
                            nc.scalar.dma_start(y_ap[:, lb, 0:h], f_t[:, lb, 0:h])
                            nc.vector.tensor_copy(
                                f_t[:, lb, h : h + q], ps_h[3][:, 0:q]
                            )
                            nc.sync.dma_start(
                                y_ap[:, lb, h : h + q], f_t[:, lb, h : h + q]
                            )
                            nc.scalar.activation(
                                f_t[:, lb, h + q : D], ps_h[3][:, q:h], Copy
                            )
                            nc.scalar.dma_start(
                                y_ap[:, lb, h + q : D], f_t[:, lb, h + q : D]
                            )
                        continue
                    if lb % 2 == 0:
                        nc.vector.tensor_copy(f_t[:, lb, :], ps_f[lb][:])
                    else:
                        nc.scalar.activation(f_t[:, lb, :], ps_f[lb][:], Copy)
                    nc.gpsimd.dma_start(y_ap[:, lb, :], f_t[:, lb, :])
    nc.compile()
    return nc


def _prepare_in_maps(x, u):
    f16 = np.float16
    in_maps = []
    for c in range(NCORES):
        ns = slice(c * NLOC, (c + 1) * NLOC)
        xs = x[:, ns, :]  # [L, NLOC, D]
        us = u[:, ns, :]  # [O, NLOC, D]
        in_maps.append(
            {
                # X^T per n: [NLOC, D, L]
                "xt": np.ascontiguousarray(xs.transpose(1, 2, 0)).astype(f16),
                # U^T per n: [NLOC, D, O]
                "ut": np.ascontiguousarray(us.transpose(1, 2, 0)).astype(f16),
                # U natural per n, pre-scaled by 0.5 (folds sigmoid's -0.5
                # via sigmoid(a)-0.5 = 0.5*tanh(a/2)): [NLOC, O, D]
                "un": (0.5 * us.transpose(1, 0, 2)).astype(f16),
            }
        )
    return in_maps


def _run(inputs, trace=False, **spmd_kwargs):
    from concourse.bass_utils import run_bass_kernel_spmd

    x = np.asarray(inputs["x"], dtype=np.float32)
    u = np.asarray(inputs["upfold"], dtype=np.float32)
    assert x.shape == (L, N, D) and u.shape == (O, N, D)

    if "nc" not in _cache:
        _cache["nc"] = _build_program()
    nc = _cache["nc"]

    in_maps = _prepare_in_maps(x, u)
    res = run_bass_kernel_spmd(
        nc, in_maps, core_ids=list(range(NCORES)), trace=trace, **spmd_kwargs
    )
    # device y is [NLOC, L, D]; transpose back while assembling [L, N, D]
    out = np.concatenate(
        [r["y"].transpose(1, 0, 2) for r in res.results], axis=1
    )
    return np.ascontiguousarray(out.astype(np.float32)), res


def kernel(**inputs) -> np.ndarray:
    out, _ = _run(inputs, trace=False)
    return out

